# revision 15
# baseline (speedup 1.0000x reference)
"""Multi-head attention forward on 8 TRN2 NeuronCores.

Problem: x[2,2048,1024] @ {Wq,Wk,Wv}[1024,1024] (+bias) -> 16 heads of 64,
softmax(QK^T/8)V per head, concat -> @Wo[1024,1024] + bo.

Sharding: tensor-parallel over d_hid. Core c owns 2 heads (128 dims):
  - computes Q^T,K^T,V^T slices [128, 4096] from full x^T
  - attention for its (2 batches x 2 heads)
  - partial out = ctx_slice @ Wo[slice_rows] -> [4096, 1024]
Host sums the 8 partials and adds bo (pure reduction, no collectives).

v5 design:
  - exp() output is fp8e4m3; attn@V runs as dual-fp8 DoubleRow matmuls.
    The two DR groups carry V_hi / V_lo (V_lo = V - fp8(V), computed on
    device) with the SAME et8 moving tile broadcast across groups, so V
    is effectively f16-precise while streaming at 0.5 cycles/column.
  - V_aug ones column (group 0) gives the softmax denominator in ctx
    row 64; group 1's aug column is zero.
  - scores stay f16 with the baseline's zero-padded-Q trick (fp8 Q/K
    measured ~2e-2 end-to-end - too close to the gate).
  - out-projection is f16 x f16, emitted per token-chunk, partials f16.
  - emission is software-pipelined: every attention kp iteration pops a
    "filler" (next batch's loads/projections/V-prep, or the previous
    chunk's out-projection) so the PE queue never heads into a stall,
    and the exp stream on the scalar engine starts ~10us into the run.
"""

import os
import numpy as np

B, S, D = 2, 2048, 1024
NCORES = 8
HSLICE = D // NCORES          # 128 = 2 heads x 64
KT_PROJ = D // 128            # 8 contraction tiles for projections
NKT = S // 128                # 16 k-tiles per batch for attention
NKP = NKT // 2                # 8 k-tile pairs per q chunk
QH = 512                      # q chunk (scores psum = 2 banks per k-pair)
NQC = S // QH                 # 4 q chunks
CH = 512                      # matmul free-dim chunk

_cache = {}


def _build():
    import concourse.bacc as bacc
    import concourse.tile as tile
    from concourse import mybir

    f32 = mybir.dt.float32
    f32r = mybir.dt.float32r
    f16 = mybir.dt.float16
    fp8 = mybir.dt.float8e4
    AF = mybir.ActivationFunctionType
    PM = mybir.MatmulPerfMode

    nc = bacc.Bacc("TRN2", target_bir_lowering=False, debug=False,
                   num_devices=NCORES)

    xt_d = nc.dram_tensor("xt", [D, B * S], f16, kind="ExternalInput").ap()
    wq_d = nc.dram_tensor("wq", [D, HSLICE], f16, kind="ExternalInput").ap()
    wk_d = nc.dram_tensor("wk", [D, HSLICE], f16, kind="ExternalInput").ap()
    wv_d = nc.dram_tensor("wv", [D, HSLICE], f16, kind="ExternalInput").ap()
    bq_d = nc.dram_tensor("bq", [HSLICE, 1], f32, kind="ExternalInput").ap()
    bk_d = nc.dram_tensor("bk", [HSLICE, 1], f32, kind="ExternalInput").ap()
    bv_d = nc.dram_tensor("bv", [HSLICE, 1], f32, kind="ExternalInput").ap()
    wo_d = nc.dram_tensor("wo", [HSLICE, D], f16, kind="ExternalInput").ap()
    idt_d = nc.dram_tensor("idt", [128, 128], f32r, kind="ExternalInput").ap()
    out_d = nc.dram_tensor("out", [B * S, D], f16, kind="ExternalOutput").ap()

    with tile.TileContext(nc) as tc:
        with (
            tc.tile_pool(name="wpool", bufs=1) as wpool,
            tc.tile_pool(name="xt", bufs=1) as xtp,
            tc.tile_pool(name="qk", bufs=2) as qkp,
            tc.tile_pool(name="vtp", bufs=2) as vtp,
            tc.tile_pool(name="vap", bufs=2) as vap,
            tc.tile_pool(name="etp", bufs=3) as etp,
            tc.tile_pool(name="ctx", bufs=2) as ctxp,
            tc.tile_pool(name="norm", bufs=2) as normp,
            tc.tile_pool(name="ost", bufs=4) as ostp,
            tc.tile_pool(name="psS", bufs=2, space="PSUM") as psS,
            tc.tile_pool(name="psC", bufs=1, space="PSUM") as psC,
            tc.tile_pool(name="psP", bufs=2, space="PSUM") as psP,
        ):
            # ---- constants / weights (issued on the gpsimd queue: the
            # scalar engine must stay exp-only) ----
            wq_t, wk_t, wv_t = [], [], []
            for ki in range(KT_PROJ):
                for lst, src, tag in ((wq_t, wq_d, "wq"), (wk_t, wk_d, "wk"),
                                      (wv_t, wv_d, "wv")):
                    t = wpool.tile([128, HSLICE], f16, tag=f"{tag}{ki}",
                                   name=f"{tag}{ki}")
                    nc.gpsimd.dma_start(t[:], src[ki * 128:(ki + 1) * 128, :])
                    lst.append(t)
            wo_t = wpool.tile([128, D], f16, tag="wo", name="wo")
            nc.gpsimd.dma_start(wo_t[:], wo_d[:])
            idt = wpool.tile([128, 128], f32r, tag="idt", name="idt")
            nc.gpsimd.dma_start(idt[:], idt_d[:])
            bq_t = wpool.tile([128, 1], f32, tag="bq", name="bq")
            nc.gpsimd.dma_start(bq_t[:], bq_d[:])
            bk_t = wpool.tile([128, 1], f32, tag="bk", name="bk")
            nc.gpsimd.dma_start(bk_t[:], bk_d[:])
            bv_t = wpool.tile([128, 1], f32, tag="bv", name="bv")
            nc.gpsimd.dma_start(bv_t[:], bv_d[:])

            xts = [None] * KT_PROJ
            cur = {}

            def emit_xt_chunk(b, c):
                s0 = b * S
                for ki in range(KT_PROJ):
                    if b == 0 and c == 0:
                        xts[ki] = xtp.tile([128, S], f16, tag=f"xt{ki}",
                                           name=f"xt{ki}")
                    nc.sync.dma_start(
                        xts[ki][:, c * CH:(c + 1) * CH],
                        xt_d[ki * 128:(ki + 1) * 128,
                             s0 + c * CH:s0 + (c + 1) * CH])

            def emit_qkv_alloc(b):
                qt0 = qkp.tile([128, S], f16, tag="qt0", name="qt0")
                qt1 = qkp.tile([128, S], f16, tag="qt1", name="qt1")
                kt = qkp.tile([128, S], f16, tag="kt", name="kt")
                vt = vtp.tile([128, S], f32r, tag="vt", name="vt")
                nc.vector.memset(qt0[64:128, :], 0.0)
                nc.vector.memset(qt1[0:64, :], 0.0)
                cur[b] = {"qth": [qt0, qt1], "kt": kt, "vt": vt, "va8": {}}

            def emit_proj_chunk(b, c, which):
                st = cur[b]
                w_t, b_t = {"q": (wq_t, bq_t), "k": (wk_t, bk_t),
                            "v": (wv_t, bv_t)}[which]
                ps = psP.tile([128, CH], f32, tag="pp", name="pp")
                for ki in range(KT_PROJ):
                    nc.tensor.matmul(ps[:], w_t[ki][:],
                                     xts[ki][:, c * CH:(c + 1) * CH],
                                     start=(ki == 0),
                                     stop=(ki == KT_PROJ - 1))
                sl = slice(c * CH, (c + 1) * CH)
                if which == "q":
                    qt0, qt1 = st["qth"]
                    nc.vector.tensor_scalar_add(qt0[0:64, sl], ps[0:64, :],
                                                b_t[0:64, 0:1])
                    nc.vector.tensor_scalar_add(qt1[64:128, sl], ps[64:128, :],
                                                b_t[64:128, 0:1])
                elif which == "k":
                    nc.vector.tensor_scalar_add(st["kt"][:, sl], ps[:],
                                                b_t[:, 0:1])
                else:
                    nc.vector.tensor_scalar_add(st["vt"][:, sl], ps[:],
                                                b_t[:, 0:1])

            def emit_vprep(b, ki):
                """Transpose V k-tile ki; store fp8 hi/lo halves augmented
                with the ones (hi) / zeros (lo) denominator column."""
                st = cur[b]
                ps = psP.tile([128, 128], f32r, tag="pp", name="ppt")
                nc.tensor.transpose(ps[:], st["vt"][:, ki * 128:(ki + 1) * 128],
                                    idt[:])
                for h in range(2):
                    va = vap.tile([128, 2, 80], fp8, tag=f"va{ki}h{h}",
                                  name=f"va{ki}h{h}")
                    st["va8"][(ki, h)] = va
                    nc.gpsimd.memset(va[:, 0, 64:65], 1.0)
                    nc.gpsimd.memset(va[:, 1, 64:65], 0.0)
                    nc.vector.tensor_copy(va[:, 0, 0:64],
                                          ps[:, h * 64:(h + 1) * 64])
                    nc.vector.tensor_sub(va[:, 1, 0:64],
                                         ps[:, h * 64:(h + 1) * 64],
                                         va[:, 0, 0:64])

            def emit_ctx(b, qh, kp, h, et8, ctx_ps):
                st = cur[b]
                for j in range(2):
                    ki = 2 * kp + j
                    va = st["va8"][(ki, h)]
                    for n in range(2):
                        rhs = (et8[:, j, n * 256:(n + 1) * 256]
                               .unsqueeze(1).broadcast_to([128, 2, 256]))
                        # start only on the first matmul touching the bank
                        # (PSUM pending-zero is bank-granular)
                        nc.tensor.matmul(
                            ctx_ps[h][:, n * 256:(n + 1) * 256],
                            va[:, :, 0:65], rhs,
                            start=(kp == 0 and j == 0 and n == 0),
                            stop=(kp == NKP - 1 and j == 1),
                            perf_mode=PM.DoubleRow,
                            skip_group_check=True)

            def emit_attn_chunk(b, qh, fillers):
                """Scores + exp + (one-kp-deferred) fp8 ctx for one q chunk,
                popping one filler emission per kp iteration; normalize at
                the end. Fillers must respect kp-indexed data deps."""
                st = cur[b]
                q0 = qh * QH
                ctx_ps = [psC.tile([65, QH], f32, tag=f"ctx{h}",
                                   name=f"ctx{h}") for h in range(2)]
                pend = []
                for kp in range(NKP):
                    if fillers:
                        fillers.pop(0)()
                    for h in range(2):
                        sc = psS.tile([128, 2 * QH], f32, tag="sc", name="sc")
                        for j in range(2):
                            ki = 2 * kp + j
                            nc.tensor.matmul(
                                sc[:, j * QH:(j + 1) * QH],
                                st["kt"][:, ki * 128:(ki + 1) * 128],
                                st["qth"][h][:, q0:q0 + QH])
                        et8 = etp.tile([128, 2, QH], fp8, tag=f"et{h}",
                                       name=f"et{h}")
                        nc.scalar.activation(et8[:, :, :], sc[:, :], AF.Exp)
                        pend.append((kp, h, et8))
                        if len(pend) > 2:
                            pkp, ph, pet = pend.pop(0)
                            emit_ctx(b, qh, pkp, ph, pet, ctx_ps)
                while fillers:
                    fillers.pop(0)()
                for pkp, ph, pet in pend:
                    emit_ctx(b, qh, pkp, ph, pet, ctx_ps)
                # normalize now - the psC banks are recycled next chunk
                ctxT = st["ctxT"]
                for h in range(2):
                    hp = h * 64
                    stg = normp.tile([65, QH], f32, tag=f"stg{h}",
                                     name=f"stg{h}")
                    nc.vector.tensor_copy(stg[0:65, :], ctx_ps[h][0:65, :])
                    r0 = normp.tile([1, QH], f32, tag="r0", name="r0")
                    nc.gpsimd.dma_start(r0[:], stg[64:65, :])
                    bcs = normp.tile([64, QH], f32, tag="bcs", name="bcs")
                    nc.gpsimd.partition_broadcast(bcs[:], r0[:])
                    bc = normp.tile([64, QH], f32, tag="bc", name="bc")
                    scr = normp.tile([64, QH], f32, tag="scr", name="scr")
                    nc.vector.reciprocal_approx_accurate(bc[:], bcs[:],
                                                         scratch=scr[:])
                    nc.vector.tensor_mul(out=ctxT[hp:hp + 64, q0:q0 + QH],
                                         in0=stg[0:64, :], in1=bc[:])

            def emit_outproj(b, qh, sts):
                st = cur[b]
                ctxT = st["ctxT"]
                s0 = b * S
                for sti in sts:
                    stt = qh * (QH // 128) + sti
                    for c in range(D // CH):
                        ps = psP.tile([128, CH], f32, tag="pp", name="ppo")
                        nc.tensor.matmul(ps[:],
                                         ctxT[:, stt * 128:(stt + 1) * 128],
                                         wo_t[:, c * CH:(c + 1) * CH])
                        ot = ostp.tile([128, CH], f16, tag="ost", name="ost")
                        nc.vector.tensor_copy(ot[:], ps[:])
                        nc.sync.dma_start(
                            out_d[s0 + stt * 128:s0 + (stt + 1) * 128,
                                  c * CH:(c + 1) * CH], ot[:])

            def vp(b, kis):
                return lambda: [emit_vprep(b, ki) for ki in kis]

            def pj(b, c, which):
                return lambda: emit_proj_chunk(b, c, which)

            def multi(*fns):
                def run():
                    for f in fns:
                        f()
                return run

            # ================= emission schedule =================
            for c in range(NQC):
                emit_xt_chunk(0, c)
            emit_qkv_alloc(0)
            for which in ("q", "k", "v"):
                emit_proj_chunk(0, 0, which)
            for ki in range(4):
                emit_vprep(0, ki)
            cur[0]["ctxT"] = ctxp.tile([128, S], f16, tag="ctxT", name="ctxT")

            # batch-0 chunk 0: fillers stream in the rest of batch 0's
            # projections and V-prep just ahead of their consumers
            emit_attn_chunk(0, 0, [
                pj(0, 1, "k"), pj(0, 1, "v"), vp(0, range(4, 8)),
                pj(0, 2, "k"), multi(pj(0, 2, "v"), vp(0, range(8, 12))),
                pj(0, 3, "k"), multi(pj(0, 3, "v"), vp(0, range(12, 16))),
                multi(pj(0, 1, "q"), pj(0, 2, "q"), pj(0, 3, "q")),
            ])
            emit_qkv_alloc(1)
            # batch-0 chunks 1-3: fillers carry the previous chunk's
            # out-projection plus batch 1's loads/proj/V-prep
            for qh in range(1, NQC):
                c = qh - 1
                emit_attn_chunk(0, qh, [
                    lambda b=0, q=c: emit_outproj(b, q, [0, 1]),
                    lambda b=0, q=c: emit_outproj(b, q, [2, 3]),
                    lambda cc=c: emit_xt_chunk(1, cc),
                    pj(1, c, "q"), pj(1, c, "k"), pj(1, c, "v"),
                    vp(1, range(4 * c, 4 * c + 2)),
                    vp(1, range(4 * c + 2, 4 * c + 4)),
                ])
            cur[1]["ctxT"] = ctxp.tile([128, S], f16, tag="ctxT", name="ctxT")

            # batch 1 chunk 0: finish batch 1 prep + batch 0's last
            # out-projection
            emit_attn_chunk(1, 0, [
                lambda: emit_xt_chunk(1, 3),
                pj(1, 3, "k"), pj(1, 3, "q"), pj(1, 3, "v"),
                vp(1, range(12, 14)), vp(1, range(14, 16)),
                lambda: emit_outproj(0, 3, [0, 1]),
                lambda: emit_outproj(0, 3, [2, 3]),
            ])
            for qh in range(1, NQC):
                c = qh - 1
                emit_attn_chunk(1, qh, [
                    lambda q=c: emit_outproj(1, q, [0, 1]),
                    lambda q=c: emit_outproj(1, q, [2, 3]),
                ])
            emit_outproj(1, NQC - 1, [0, 1, 2, 3])

    nc.compile()
    return nc


def _get_nc():
    if "nc" not in _cache:
        _cache["nc"] = _build()
    return _cache["nc"]


def kernel(x, Wq, bq, Wk, bk, Wv, bv, Wo, bo):
    from concourse.bass_utils import run_bass_kernel_spmd

    nc = _get_nc()

    x = np.ascontiguousarray(np.asarray(x, dtype=np.float32))
    xt = np.ascontiguousarray(x.reshape(B * S, D).T)          # [D, B*S]
    idt = np.eye(128, dtype=np.float32)

    in_maps = []
    for c in range(NCORES):
        sl = slice(c * HSLICE, (c + 1) * HSLICE)
        in_maps.append({
            "xt": xt.astype(np.float16),
            "wq": (np.ascontiguousarray(np.asarray(Wq, np.float32)[:, sl]) / 8.0).astype(np.float16),
            "wk": np.ascontiguousarray(np.asarray(Wk, np.float32)[:, sl]).astype(np.float16),
            "wv": np.ascontiguousarray(np.asarray(Wv, np.float32)[:, sl]).astype(np.float16),
            "bq": (np.asarray(bq, np.float32)[sl] / 8.0).reshape(HSLICE, 1),
            "bk": np.asarray(bk, np.float32)[sl].reshape(HSLICE, 1),
            "bv": np.asarray(bv, np.float32)[sl].reshape(HSLICE, 1),
            "wo": np.ascontiguousarray(np.asarray(Wo, np.float32)[sl, :]).astype(np.float16),
            "idt": idt,
        })

    res = run_bass_kernel_spmd(nc, in_maps, core_ids=list(range(NCORES)),
                               trace=bool(int(os.environ.get("KTRACE", "0"))))
    _cache["last_result"] = res
    acc = res.results[0]["out"].astype(np.float32)
    for c in range(1, NCORES):
        acc += res.results[c]["out"].astype(np.float32)
    acc += np.asarray(bo, np.float32)[None, :]
    return acc.reshape(B, S, D)


# revision 22
# speedup vs baseline: 1.0585x; 1.0585x over previous
"""Multi-head attention forward on 8 TRN2 NeuronCores.

Problem: x[2,2048,1024] @ {Wq,Wk,Wv}[1024,1024] (+bias) -> 16 heads of 64,
softmax(QK^T/8)V per head, concat -> @Wo[1024,1024] + bo.

Sharding: tensor-parallel over d_hid. Core c owns 2 heads (128 dims):
  - computes Q^T,K^T,V^T slices [128, 4096] from full x^T
  - attention for its (2 batches x 2 heads)
  - partial out = ctx_slice @ Wo[slice_rows] -> [4096, 1024]
Host sums the 8 partials and adds bo (pure reduction, no collectives).

v5 design:
  - exp() output is fp8e4m3; attn@V runs as dual-fp8 DoubleRow matmuls.
    The two DR groups carry V_hi / V_lo (V_lo = V - fp8(V), computed on
    device) with the SAME et8 moving tile broadcast across groups, so V
    is effectively f16-precise while streaming at 0.5 cycles/column.
  - V_aug ones column (group 0) gives the softmax denominator in ctx
    row 64; group 1's aug column is zero.
  - scores stay f16 with the baseline's zero-padded-Q trick (fp8 Q/K
    measured ~2e-2 end-to-end - too close to the gate).
  - out-projection is f16 x f16, emitted per token-chunk, partials f16.
  - emission is software-pipelined: every attention kp iteration pops a
    "filler" (next batch's loads/projections/V-prep, or the previous
    chunk's out-projection) so the PE queue never heads into a stall,
    and the exp stream on the scalar engine starts ~10us into the run.
"""

import os
import numpy as np

B, S, D = 2, 2048, 1024
NCORES = 8
HSLICE = D // NCORES          # 128 = 2 heads x 64
KT_PROJ = D // 128            # 8 contraction tiles for projections
NKT = S // 128                # 16 k-tiles per batch for attention
NKP = NKT // 2                # 8 k-tile pairs per q chunk
QH = 512                      # q chunk (scores psum = 2 banks per k-pair)
NQC = S // QH                 # 4 q chunks
CH = 512                      # matmul free-dim chunk

_cache = {}


def _build():
    import concourse.bacc as bacc
    import concourse.tile as tile
    from concourse import mybir

    f32 = mybir.dt.float32
    f32r = mybir.dt.float32r
    f16 = mybir.dt.float16
    fp8 = mybir.dt.float8e4
    AF = mybir.ActivationFunctionType
    PM = mybir.MatmulPerfMode

    nc = bacc.Bacc("TRN2", target_bir_lowering=False, debug=False,
                   num_devices=NCORES)

    xt_d = nc.dram_tensor("xt", [D, B * S], f16, kind="ExternalInput").ap()
    # weights pre-transposed on host to [128, KT_PROJ, 128] so each loads
    # with a single contiguous DMA
    wq_d = nc.dram_tensor("wq", [128, KT_PROJ, HSLICE], f16,
                          kind="ExternalInput").ap()
    wk_d = nc.dram_tensor("wk", [128, KT_PROJ, HSLICE], f16,
                          kind="ExternalInput").ap()
    wv_d = nc.dram_tensor("wv", [128, KT_PROJ, HSLICE], f16,
                          kind="ExternalInput").ap()
    bq_d = nc.dram_tensor("bq", [HSLICE, 1], f32, kind="ExternalInput").ap()
    bk_d = nc.dram_tensor("bk", [HSLICE, 1], f32, kind="ExternalInput").ap()
    bv_d = nc.dram_tensor("bv", [HSLICE, 1], f32, kind="ExternalInput").ap()
    wo_d = nc.dram_tensor("wo", [HSLICE, D], f16, kind="ExternalInput").ap()
    idt_d = nc.dram_tensor("idt", [128, 128], f32r, kind="ExternalInput").ap()
    out_d = nc.dram_tensor("out", [B * S, D], f16, kind="ExternalOutput").ap()

    with tile.TileContext(nc) as tc:
        with (
            tc.tile_pool(name="wpool", bufs=1) as wpool,
            tc.tile_pool(name="xt", bufs=1) as xtp,
            tc.tile_pool(name="qk", bufs=2) as qkp,
            tc.tile_pool(name="vtp", bufs=2) as vtp,
            tc.tile_pool(name="vap", bufs=2) as vap,
            tc.tile_pool(name="etp", bufs=3) as etp,
            tc.tile_pool(name="ctx", bufs=2) as ctxp,
            tc.tile_pool(name="norm", bufs=2) as normp,
            tc.tile_pool(name="ost", bufs=4) as ostp,
            tc.tile_pool(name="psS", bufs=2, space="PSUM") as psS,
            tc.tile_pool(name="psC", bufs=1, space="PSUM") as psC,
            tc.tile_pool(name="psP", bufs=2, space="PSUM") as psP,
        ):
            # ---- constants / weights: one contiguous DMA per tensor so
            # the first projection's weights land within a few us (the
            # scalar engine stays exp-only) ----
            wq_t = wpool.tile([128, KT_PROJ, HSLICE], f16, tag="wq", name="wq")
            nc.sync.dma_start(wq_t[:], wq_d[:])
            wk_t = wpool.tile([128, KT_PROJ, HSLICE], f16, tag="wk", name="wk")
            nc.sync.dma_start(wk_t[:], wk_d[:])
            wv_t = wpool.tile([128, KT_PROJ, HSLICE], f16, tag="wv", name="wv")
            nc.sync.dma_start(wv_t[:], wv_d[:])
            wo_t = wpool.tile([128, D], f16, tag="wo", name="wo")
            nc.gpsimd.dma_start(wo_t[:], wo_d[:])
            idt = wpool.tile([128, 128], f32r, tag="idt", name="idt")
            nc.gpsimd.dma_start(idt[:], idt_d[:])
            bq_t = wpool.tile([128, 1], f32, tag="bq", name="bq")
            nc.gpsimd.dma_start(bq_t[:], bq_d[:])
            bk_t = wpool.tile([128, 1], f32, tag="bk", name="bk")
            nc.gpsimd.dma_start(bk_t[:], bk_d[:])
            bv_t = wpool.tile([128, 1], f32, tag="bv", name="bv")
            nc.gpsimd.dma_start(bv_t[:], bv_d[:])

            xts = [None] * KT_PROJ
            cur = {}

            def emit_xt_chunk(b, c):
                s0 = b * S
                for ki in range(KT_PROJ):
                    if b == 0 and c == 0:
                        xts[ki] = xtp.tile([128, S], f16, tag=f"xt{ki}",
                                           name=f"xt{ki}")
                    nc.sync.dma_start(
                        xts[ki][:, c * CH:(c + 1) * CH],
                        xt_d[ki * 128:(ki + 1) * 128,
                             s0 + c * CH:s0 + (c + 1) * CH])

            def emit_qkv_alloc(b):
                qt0 = qkp.tile([128, S], f16, tag="qt0", name="qt0")
                qt1 = qkp.tile([128, S], f16, tag="qt1", name="qt1")
                kt = qkp.tile([128, S], f16, tag="kt", name="kt")
                vt = vtp.tile([128, S], f32r, tag="vt", name="vt")
                nc.vector.memset(qt0[64:128, :], 0.0)
                nc.vector.memset(qt1[0:64, :], 0.0)
                cur[b] = {"qth": [qt0, qt1], "kt": kt, "vt": vt, "va8": {}}

            def emit_proj_chunk(b, c, which):
                st = cur[b]
                w_t, b_t = {"q": (wq_t, bq_t), "k": (wk_t, bk_t),
                            "v": (wv_t, bv_t)}[which]
                ps = psP.tile([128, CH], f32, tag="pp", name="pp")
                for ki in range(KT_PROJ):
                    nc.tensor.matmul(ps[:], w_t[:, ki, :],
                                     xts[ki][:, c * CH:(c + 1) * CH],
                                     start=(ki == 0),
                                     stop=(ki == KT_PROJ - 1))
                sl = slice(c * CH, (c + 1) * CH)
                if which == "q":
                    qt0, qt1 = st["qth"]
                    nc.vector.tensor_scalar_add(qt0[0:64, sl], ps[0:64, :],
                                                b_t[0:64, 0:1])
                    nc.vector.tensor_scalar_add(qt1[64:128, sl], ps[64:128, :],
                                                b_t[64:128, 0:1])
                elif which == "k":
                    nc.vector.tensor_scalar_add(st["kt"][:, sl], ps[:],
                                                b_t[:, 0:1])
                else:
                    nc.vector.tensor_scalar_add(st["vt"][:, sl], ps[:],
                                                b_t[:, 0:1])

            def emit_vprep(b, ki):
                """Transpose V k-tile ki; store fp8 hi/lo halves augmented
                with the ones (hi) / zeros (lo) denominator column."""
                st = cur[b]
                ps = psP.tile([128, 128], f32r, tag="pp", name="ppt")
                nc.tensor.transpose(ps[:], st["vt"][:, ki * 128:(ki + 1) * 128],
                                    idt[:])
                for h in range(2):
                    va = vap.tile([128, 2, 80], fp8, tag=f"va{ki}h{h}",
                                  name=f"va{ki}h{h}")
                    st["va8"][(ki, h)] = va
                    nc.gpsimd.memset(va[:, 0, 64:65], 1.0)
                    nc.gpsimd.memset(va[:, 1, 64:65], 0.0)
                    nc.vector.tensor_copy(va[:, 0, 0:64],
                                          ps[:, h * 64:(h + 1) * 64])
                    nc.vector.tensor_sub(va[:, 1, 0:64],
                                         ps[:, h * 64:(h + 1) * 64],
                                         va[:, 0, 0:64])

            def emit_ctx(b, qh, kp, h, et8, ctx_ps):
                st = cur[b]
                for j in range(2):
                    ki = 2 * kp + j
                    va = st["va8"][(ki, h)]
                    for n in range(2):
                        rhs = (et8[:, j, n * 256:(n + 1) * 256]
                               .unsqueeze(1).broadcast_to([128, 2, 256]))
                        # start only on the first matmul touching the bank
                        # (PSUM pending-zero is bank-granular)
                        nc.tensor.matmul(
                            ctx_ps[h][:, n * 256:(n + 1) * 256],
                            va[:, :, 0:65], rhs,
                            start=(kp == 0 and j == 0 and n == 0),
                            stop=(kp == NKP - 1 and j == 1),
                            perf_mode=PM.DoubleRow,
                            skip_group_check=True)

            def emit_attn_chunk(b, qh, fillers):
                """Scores + exp + (one-kp-deferred) fp8 ctx for one q chunk,
                popping one filler emission per kp iteration; normalize at
                the end. Fillers must respect kp-indexed data deps."""
                st = cur[b]
                q0 = qh * QH
                ctx_ps = [psC.tile([65, QH], f32, tag=f"ctx{h}",
                                   name=f"ctx{h}") for h in range(2)]
                pend = []
                for kp in range(NKP):
                    if fillers:
                        fillers.pop(0)()
                    # ktile-major: both heads' scores against one k-tile
                    # share the PE stationary (LDWEIGHTS elides)
                    scs = [psS.tile([128, 2 * QH], f32, tag="sc", name="sc")
                           for _ in range(2)]
                    for j in range(2):
                        ki = 2 * kp + j
                        for h in range(2):
                            nc.tensor.matmul(
                                scs[h][:, j * QH:(j + 1) * QH],
                                st["kt"][:, ki * 128:(ki + 1) * 128],
                                st["qth"][h][:, q0:q0 + QH])
                    for h in range(2):
                        et8 = etp.tile([128, 2, QH], fp8, tag=f"et{h}",
                                       name=f"et{h}")
                        nc.scalar.activation(et8[:, :, :], scs[h][:, :], AF.Exp)
                        pend.append((kp, h, et8))
                        if len(pend) > 2:
                            pkp, ph, pet = pend.pop(0)
                            emit_ctx(b, qh, pkp, ph, pet, ctx_ps)
                while fillers:
                    fillers.pop(0)()
                for pkp, ph, pet in pend:
                    emit_ctx(b, qh, pkp, ph, pet, ctx_ps)
                # normalize now - the psC banks are recycled next chunk
                ctxT = st["ctxT"]
                for h in range(2):
                    hp = h * 64
                    stg = normp.tile([65, QH], f32, tag=f"stg{h}",
                                     name=f"stg{h}")
                    nc.vector.tensor_copy(stg[0:65, :], ctx_ps[h][0:65, :])
                    r0 = normp.tile([1, QH], f32, tag="r0", name="r0")
                    nc.gpsimd.dma_start(r0[:], stg[64:65, :])
                    bcs = normp.tile([64, QH], f32, tag="bcs", name="bcs")
                    nc.gpsimd.partition_broadcast(bcs[:], r0[:])
                    bc = normp.tile([64, QH], f32, tag="bc", name="bc")
                    scr = normp.tile([64, QH], f32, tag="scr", name="scr")
                    nc.vector.reciprocal_approx_accurate(bc[:], bcs[:],
                                                         scratch=scr[:])
                    nc.vector.tensor_mul(out=ctxT[hp:hp + 64, q0:q0 + QH],
                                         in0=stg[0:64, :], in1=bc[:])

            def emit_outproj(b, qh, sts):
                st = cur[b]
                ctxT = st["ctxT"]
                s0 = b * S
                for sti in sts:
                    stt = qh * (QH // 128) + sti
                    for c in range(D // CH):
                        ps = psP.tile([128, CH], f32, tag="pp", name="ppo")
                        nc.tensor.matmul(ps[:],
                                         ctxT[:, stt * 128:(stt + 1) * 128],
                                         wo_t[:, c * CH:(c + 1) * CH])
                        ot = ostp.tile([128, CH], f16, tag="ost", name="ost")
                        nc.vector.tensor_copy(ot[:], ps[:])
                        nc.sync.dma_start(
                            out_d[s0 + stt * 128:s0 + (stt + 1) * 128,
                                  c * CH:(c + 1) * CH], ot[:])

            def vp(b, kis):
                return lambda: [emit_vprep(b, ki) for ki in kis]

            def pj(b, c, which):
                return lambda: emit_proj_chunk(b, c, which)

            def multi(*fns):
                def run():
                    for f in fns:
                        f()
                return run

            # ================= emission schedule =================
            for c in range(NQC):
                emit_xt_chunk(0, c)
            emit_qkv_alloc(0)
            for which in ("q", "k", "v"):
                emit_proj_chunk(0, 0, which)
            for ki in range(4):
                emit_vprep(0, ki)
            cur[0]["ctxT"] = ctxp.tile([128, S], f16, tag="ctxT", name="ctxT")

            # batch-0 chunk 0: fillers stream in the rest of batch 0's
            # projections and V-prep just ahead of their consumers
            emit_attn_chunk(0, 0, [
                pj(0, 1, "k"), pj(0, 1, "v"), vp(0, range(4, 8)),
                pj(0, 2, "k"), multi(pj(0, 2, "v"), vp(0, range(8, 12))),
                pj(0, 3, "k"), multi(pj(0, 3, "v"), vp(0, range(12, 16))),
                multi(pj(0, 1, "q"), pj(0, 2, "q"), pj(0, 3, "q")),
            ])
            emit_qkv_alloc(1)
            # batch-0 chunks 1-3: fillers carry the previous chunk's
            # out-projection plus batch 1's loads/proj/V-prep
            for qh in range(1, NQC):
                c = qh - 1
                emit_attn_chunk(0, qh, [
                    lambda b=0, q=c: emit_outproj(b, q, [0, 1]),
                    lambda b=0, q=c: emit_outproj(b, q, [2, 3]),
                    lambda cc=c: emit_xt_chunk(1, cc),
                    pj(1, c, "q"), pj(1, c, "k"), pj(1, c, "v"),
                    vp(1, range(4 * c, 4 * c + 2)),
                    vp(1, range(4 * c + 2, 4 * c + 4)),
                ])
            cur[1]["ctxT"] = ctxp.tile([128, S], f16, tag="ctxT", name="ctxT")

            # batch 1 chunk 0: finish batch 1 prep + batch 0's last
            # out-projection
            emit_attn_chunk(1, 0, [
                lambda: emit_xt_chunk(1, 3),
                pj(1, 3, "k"), pj(1, 3, "q"), pj(1, 3, "v"),
                vp(1, range(12, 14)), vp(1, range(14, 16)),
                lambda: emit_outproj(0, 3, [0, 1]),
                lambda: emit_outproj(0, 3, [2, 3]),
            ])
            for qh in range(1, NQC):
                c = qh - 1
                emit_attn_chunk(1, qh, [
                    lambda q=c: emit_outproj(1, q, [0, 1]),
                    lambda q=c: emit_outproj(1, q, [2, 3]),
                ])
            emit_outproj(1, NQC - 1, [0, 1, 2, 3])

    nc.compile()
    return nc


def _get_nc():
    if "nc" not in _cache:
        _cache["nc"] = _build()
    return _cache["nc"]


def kernel(x, Wq, bq, Wk, bk, Wv, bv, Wo, bo):
    from concourse.bass_utils import run_bass_kernel_spmd

    nc = _get_nc()

    x = np.ascontiguousarray(np.asarray(x, dtype=np.float32))
    xt = np.ascontiguousarray(x.reshape(B * S, D).T)          # [D, B*S]
    idt = np.eye(128, dtype=np.float32)

    def wprep(W, sl, scale=1.0):
        # [1024, 128] slice -> [128 part, KT_PROJ, 128] contiguous f16
        w = np.asarray(W, np.float32)[:, sl] * scale
        return np.ascontiguousarray(
            w.reshape(KT_PROJ, 128, HSLICE).transpose(1, 0, 2)).astype(np.float16)

    in_maps = []
    for c in range(NCORES):
        sl = slice(c * HSLICE, (c + 1) * HSLICE)
        in_maps.append({
            "xt": xt.astype(np.float16),
            "wq": wprep(Wq, sl, 1.0 / 8.0),
            "wk": wprep(Wk, sl),
            "wv": wprep(Wv, sl),
            "bq": (np.asarray(bq, np.float32)[sl] / 8.0).reshape(HSLICE, 1),
            "bk": np.asarray(bk, np.float32)[sl].reshape(HSLICE, 1),
            "bv": np.asarray(bv, np.float32)[sl].reshape(HSLICE, 1),
            "wo": np.ascontiguousarray(np.asarray(Wo, np.float32)[sl, :]).astype(np.float16),
            "idt": idt,
        })

    res = run_bass_kernel_spmd(nc, in_maps, core_ids=list(range(NCORES)),
                               trace=bool(int(os.environ.get("KTRACE", "0"))))
    _cache["last_result"] = res
    acc = res.results[0]["out"].astype(np.float32)
    for c in range(1, NCORES):
        acc += res.results[c]["out"].astype(np.float32)
    acc += np.asarray(bo, np.float32)[None, :]
    return acc.reshape(B, S, D)


# revision 23
# speedup vs baseline: 1.0796x; 1.0199x over previous
"""Multi-head attention forward on 8 TRN2 NeuronCores.

Problem: x[2,2048,1024] @ {Wq,Wk,Wv}[1024,1024] (+bias) -> 16 heads of 64,
softmax(QK^T/8)V per head, concat -> @Wo[1024,1024] + bo.

Sharding: tensor-parallel over d_hid. Core c owns 2 heads (128 dims):
  - computes Q^T,K^T,V^T slices [128, 4096] from full x^T
  - attention for its (2 batches x 2 heads)
  - partial out = ctx_slice @ Wo[slice_rows] -> [4096, 1024]
Host sums the 8 partials and adds bo (pure reduction, no collectives).

v5 design:
  - exp() output is fp8e4m3; attn@V runs as dual-fp8 DoubleRow matmuls.
    The two DR groups carry V_hi / V_lo (V_lo = V - fp8(V), computed on
    device) with the SAME et8 moving tile broadcast across groups, so V
    is effectively f16-precise while streaming at 0.5 cycles/column.
  - V_aug ones column (group 0) gives the softmax denominator in ctx
    row 64; group 1's aug column is zero.
  - scores stay f16 with the baseline's zero-padded-Q trick (fp8 Q/K
    measured ~2e-2 end-to-end - too close to the gate).
  - out-projection is f16 x f16, emitted per token-chunk, partials f16.
  - emission is software-pipelined: every attention kp iteration pops a
    "filler" (next batch's loads/projections/V-prep, or the previous
    chunk's out-projection) so the PE queue never heads into a stall,
    and the exp stream on the scalar engine starts ~10us into the run.
"""

import os
import numpy as np

B, S, D = 2, 2048, 1024
NCORES = 8
HSLICE = D // NCORES          # 128 = 2 heads x 64
KT_PROJ = D // 128            # 8 contraction tiles for projections
NKT = S // 128                # 16 k-tiles per batch for attention
NKP = NKT // 2                # 8 k-tile pairs per q chunk
QH = 512                      # q chunk (scores psum = 2 banks per k-pair)
NQC = S // QH                 # 4 q chunks
CH = 512                      # matmul free-dim chunk

_cache = {}


def _build():
    import concourse.bacc as bacc
    import concourse.tile as tile
    from concourse import mybir

    f32 = mybir.dt.float32
    f32r = mybir.dt.float32r
    f16 = mybir.dt.float16
    fp8 = mybir.dt.float8e4
    AF = mybir.ActivationFunctionType
    PM = mybir.MatmulPerfMode

    nc = bacc.Bacc("TRN2", target_bir_lowering=False, debug=False,
                   num_devices=NCORES)

    xt_d = nc.dram_tensor("xt", [D, B * S], f16, kind="ExternalInput").ap()
    # weights pre-transposed on host to [128, KT_PROJ, 128] so each loads
    # with a single contiguous DMA
    wq_d = nc.dram_tensor("wq", [128, KT_PROJ, HSLICE], f16,
                          kind="ExternalInput").ap()
    wk_d = nc.dram_tensor("wk", [128, KT_PROJ, HSLICE], f16,
                          kind="ExternalInput").ap()
    wv_d = nc.dram_tensor("wv", [128, KT_PROJ, HSLICE], f16,
                          kind="ExternalInput").ap()
    bq_d = nc.dram_tensor("bq", [HSLICE, 1], f32, kind="ExternalInput").ap()
    bk_d = nc.dram_tensor("bk", [HSLICE, 1], f32, kind="ExternalInput").ap()
    bv_d = nc.dram_tensor("bv", [HSLICE, 1], f32, kind="ExternalInput").ap()
    wo_d = nc.dram_tensor("wo", [HSLICE, D], f16, kind="ExternalInput").ap()
    idt_d = nc.dram_tensor("idt", [128, 128], f32r, kind="ExternalInput").ap()
    out_d = nc.dram_tensor("out", [B * S, D], f16, kind="ExternalOutput").ap()

    with tile.TileContext(nc) as tc:
        with (
            tc.tile_pool(name="wpool", bufs=1) as wpool,
            tc.tile_pool(name="xt", bufs=1) as xtp,
            tc.tile_pool(name="qk", bufs=2) as qkp,
            tc.tile_pool(name="vtp", bufs=2) as vtp,
            tc.tile_pool(name="vap", bufs=2) as vap,
            tc.tile_pool(name="etp", bufs=3) as etp,
            tc.tile_pool(name="ctx", bufs=2) as ctxp,
            tc.tile_pool(name="norm", bufs=2) as normp,
            tc.tile_pool(name="ost", bufs=4) as ostp,
            tc.tile_pool(name="psS", bufs=2, space="PSUM") as psS,
            tc.tile_pool(name="psC", bufs=1, space="PSUM") as psC,
            tc.tile_pool(name="psP", bufs=2, space="PSUM") as psP,
        ):
            # ---- constants / weights: one contiguous DMA per tensor so
            # the first projection's weights land within a few us (the
            # scalar engine stays exp-only) ----
            wq_t = wpool.tile([128, KT_PROJ, HSLICE], f16, tag="wq", name="wq")
            nc.sync.dma_start(wq_t[:], wq_d[:])
            wk_t = wpool.tile([128, KT_PROJ, HSLICE], f16, tag="wk", name="wk")
            nc.sync.dma_start(wk_t[:], wk_d[:])
            wv_t = wpool.tile([128, KT_PROJ, HSLICE], f16, tag="wv", name="wv")
            nc.sync.dma_start(wv_t[:], wv_d[:])
            wo_t = wpool.tile([128, D], f16, tag="wo", name="wo")
            nc.gpsimd.dma_start(wo_t[:], wo_d[:])
            idt = wpool.tile([128, 128], f32r, tag="idt", name="idt")
            nc.gpsimd.dma_start(idt[:], idt_d[:])
            bq_t = wpool.tile([128, 1], f32, tag="bq", name="bq")
            nc.gpsimd.dma_start(bq_t[:], bq_d[:])
            bk_t = wpool.tile([128, 1], f32, tag="bk", name="bk")
            nc.gpsimd.dma_start(bk_t[:], bk_d[:])
            bv_t = wpool.tile([128, 1], f32, tag="bv", name="bv")
            nc.gpsimd.dma_start(bv_t[:], bv_d[:])

            xts = [None] * KT_PROJ
            cur = {}

            def emit_xt_chunk(b, c):
                s0 = b * S
                for ki in range(KT_PROJ):
                    if b == 0 and c == 0:
                        xts[ki] = xtp.tile([128, S], f16, tag=f"xt{ki}",
                                           name=f"xt{ki}")
                    nc.sync.dma_start(
                        xts[ki][:, c * CH:(c + 1) * CH],
                        xt_d[ki * 128:(ki + 1) * 128,
                             s0 + c * CH:s0 + (c + 1) * CH])

            def emit_qkv_alloc(b):
                qt0 = qkp.tile([128, S], f16, tag="qt0", name="qt0")
                qt1 = qkp.tile([128, S], f16, tag="qt1", name="qt1")
                kt = qkp.tile([128, S], f16, tag="kt", name="kt")
                vt = vtp.tile([128, S], f32r, tag="vt", name="vt")
                nc.vector.memset(qt0[64:128, :], 0.0)
                nc.vector.memset(qt1[0:64, :], 0.0)
                cur[b] = {"qth": [qt0, qt1], "kt": kt, "vt": vt, "va8": {}}

            def emit_proj_chunk(b, c, which):
                st = cur[b]
                w_t, b_t = {"q": (wq_t, bq_t), "k": (wk_t, bk_t),
                            "v": (wv_t, bv_t)}[which]
                ps = psP.tile([128, CH], f32, tag="pp", name="pp")
                for ki in range(KT_PROJ):
                    nc.tensor.matmul(ps[:], w_t[:, ki, :],
                                     xts[ki][:, c * CH:(c + 1) * CH],
                                     start=(ki == 0),
                                     stop=(ki == KT_PROJ - 1))
                sl = slice(c * CH, (c + 1) * CH)
                if which == "q":
                    qt0, qt1 = st["qth"]
                    nc.vector.tensor_scalar_add(qt0[0:64, sl], ps[0:64, :],
                                                b_t[0:64, 0:1])
                    nc.vector.tensor_scalar_add(qt1[64:128, sl], ps[64:128, :],
                                                b_t[64:128, 0:1])
                elif which == "k":
                    nc.vector.tensor_scalar_add(st["kt"][:, sl], ps[:],
                                                b_t[:, 0:1])
                else:
                    nc.vector.tensor_scalar_add(st["vt"][:, sl], ps[:],
                                                b_t[:, 0:1])

            def emit_vprep(b, ki):
                """Transpose V k-tile ki; store fp8 hi/lo halves augmented
                with the ones (hi) / zeros (lo) denominator column."""
                st = cur[b]
                ps = psP.tile([128, 128], f32r, tag="pp", name="ppt")
                nc.tensor.transpose(ps[:], st["vt"][:, ki * 128:(ki + 1) * 128],
                                    idt[:])
                for h in range(2):
                    va = vap.tile([128, 2, 80], fp8, tag=f"va{ki}h{h}",
                                  name=f"va{ki}h{h}")
                    st["va8"][(ki, h)] = va
                    nc.gpsimd.memset(va[:, 0, 64:65], 1.0)
                    nc.gpsimd.memset(va[:, 1, 64:65], 0.0)
                    nc.vector.tensor_copy(va[:, 0, 0:64],
                                          ps[:, h * 64:(h + 1) * 64])
                    nc.vector.tensor_sub(va[:, 1, 0:64],
                                         ps[:, h * 64:(h + 1) * 64],
                                         va[:, 0, 0:64])

            def emit_ctx(b, qh, kp, h, et8, ctx_ps):
                st = cur[b]
                for j in range(2):
                    ki = 2 * kp + j
                    va = st["va8"][(ki, h)]
                    for n in range(2):
                        rhs = (et8[:, j, n * 256:(n + 1) * 256]
                               .unsqueeze(1).broadcast_to([128, 2, 256]))
                        # start only on the first matmul touching the bank
                        # (PSUM pending-zero is bank-granular)
                        nc.tensor.matmul(
                            ctx_ps[h][:, n * 256:(n + 1) * 256],
                            va[:, :, 0:65], rhs,
                            start=(kp == 0 and j == 0 and n == 0),
                            stop=(kp == NKP - 1 and j == 1),
                            perf_mode=PM.DoubleRow,
                            skip_group_check=True)

            def emit_attn_chunk(b, qh, fillers):
                """Scores + exp + (one-kp-deferred) fp8 ctx for one q chunk,
                popping one filler emission per kp iteration; normalize at
                the end. Fillers must respect kp-indexed data deps."""
                st = cur[b]
                q0 = qh * QH
                ctx_ps = [psC.tile([65, QH], f32, tag=f"ctx{h}",
                                   name=f"ctx{h}") for h in range(2)]
                pend = []
                for kp in range(NKP):
                    if fillers:
                        fillers.pop(0)()
                    # ktile-major: both heads' scores against one k-tile
                    # share the PE stationary (LDWEIGHTS elides)
                    scs = [psS.tile([128, 2 * QH], f32, tag="sc", name="sc")
                           for _ in range(2)]
                    for j in range(2):
                        ki = 2 * kp + j
                        for h in range(2):
                            nc.tensor.matmul(
                                scs[h][:, j * QH:(j + 1) * QH],
                                st["kt"][:, ki * 128:(ki + 1) * 128],
                                st["qth"][h][:, q0:q0 + QH])
                    for h in range(2):
                        et8 = etp.tile([128, 2, QH], fp8, tag=f"et{h}",
                                       name=f"et{h}")
                        nc.scalar.activation(et8[:, :, :], scs[h][:, :], AF.Exp)
                        pend.append((kp, h, et8))
                        if len(pend) > 2:
                            pkp, ph, pet = pend.pop(0)
                            emit_ctx(b, qh, pkp, ph, pet, ctx_ps)
                while fillers:
                    fillers.pop(0)()
                for pkp, ph, pet in pend:
                    emit_ctx(b, qh, pkp, ph, pet, ctx_ps)
                # normalize now - the psC banks are recycled next chunk
                ctxT = st["ctxT"]
                for h in range(2):
                    hp = h * 64
                    stg = normp.tile([65, QH], f32, tag=f"stg{h}",
                                     name=f"stg{h}")
                    nc.vector.tensor_copy(stg[0:65, :], ctx_ps[h][0:65, :])
                    r0 = normp.tile([1, QH], f32, tag="r0", name="r0")
                    nc.gpsimd.dma_start(r0[:], stg[64:65, :])
                    bcs = normp.tile([64, QH], f32, tag="bcs", name="bcs")
                    nc.gpsimd.partition_broadcast(bcs[:], r0[:])
                    bc = normp.tile([64, QH], f32, tag="bc", name="bc")
                    scr = normp.tile([64, QH], f32, tag="scr", name="scr")
                    nc.vector.reciprocal_approx_accurate(bc[:], bcs[:],
                                                         scratch=scr[:])
                    nc.vector.tensor_mul(out=ctxT[hp:hp + 64, q0:q0 + QH],
                                         in0=stg[0:64, :], in1=bc[:])

            def emit_outproj(b, qh, sts):
                st = cur[b]
                ctxT = st["ctxT"]
                s0 = b * S
                for sti in sts:
                    stt = qh * (QH // 128) + sti
                    for c in range(D // CH):
                        ps = psP.tile([128, CH], f32, tag="pp", name="ppo")
                        nc.tensor.matmul(ps[:],
                                         ctxT[:, stt * 128:(stt + 1) * 128],
                                         wo_t[:, c * CH:(c + 1) * CH])
                        ot = ostp.tile([128, CH], f16, tag="ost", name="ost")
                        nc.vector.tensor_copy(ot[:], ps[:])
                        nc.sync.dma_start(
                            out_d[s0 + stt * 128:s0 + (stt + 1) * 128,
                                  c * CH:(c + 1) * CH], ot[:])

            def vp(b, kis):
                return lambda: [emit_vprep(b, ki) for ki in kis]

            def pj(b, c, which):
                return lambda: emit_proj_chunk(b, c, which)

            def multi(*fns):
                def run():
                    for f in fns:
                        f()
                return run

            # ================= emission schedule =================
            for c in range(NQC):
                emit_xt_chunk(0, c)
            emit_qkv_alloc(0)
            # only q/k chunk 0 ahead of the first scores; everything else
            # streams in as fillers so the exp pipeline starts ~12us in
            emit_proj_chunk(0, 0, "k")
            emit_proj_chunk(0, 0, "q")
            cur[0]["ctxT"] = ctxp.tile([128, S], f16, tag="ctxT", name="ctxT")

            emit_attn_chunk(0, 0, [
                multi(pj(0, 0, "v"), vp(0, range(0, 2))),
                vp(0, range(2, 4)),
                pj(0, 1, "k"), multi(pj(0, 1, "v"), vp(0, range(4, 8))),
                pj(0, 2, "k"), multi(pj(0, 2, "v"), vp(0, range(8, 12))),
                multi(pj(0, 3, "k"), pj(0, 3, "v"), vp(0, range(12, 16))),
                multi(pj(0, 1, "q"), pj(0, 2, "q"), pj(0, 3, "q")),
            ])
            emit_qkv_alloc(1)
            # batch-0 chunks 1-3 carry batch 1's loads/proj/V-prep; batch
            # 0's out-projections are pushed into batch 1's (lighter)
            # chunks to balance PE load per exp window
            for qh in range(1, NQC):
                c = qh - 1
                emit_attn_chunk(0, qh, [
                    lambda cc=c: emit_xt_chunk(1, cc),
                    pj(1, c, "q"), pj(1, c, "k"), pj(1, c, "v"),
                    vp(1, range(4 * c, 4 * c + 2)),
                    vp(1, range(4 * c + 2, 4 * c + 4)),
                ])
            cur[1]["ctxT"] = ctxp.tile([128, S], f16, tag="ctxT", name="ctxT")

            emit_attn_chunk(1, 0, [
                lambda: emit_xt_chunk(1, 3),
                pj(1, 3, "k"), pj(1, 3, "q"), pj(1, 3, "v"),
                vp(1, range(12, 14)), vp(1, range(14, 16)),
                lambda: emit_outproj(0, 0, [0, 1]),
                lambda: emit_outproj(0, 0, [2, 3]),
            ])
            for qh in range(1, NQC):
                c = qh - 1
                emit_attn_chunk(1, qh, [
                    lambda q=qh: emit_outproj(0, q, [0, 1]),
                    lambda q=qh: emit_outproj(0, q, [2, 3]),
                    lambda q=c: emit_outproj(1, q, [0, 1]),
                    lambda q=c: emit_outproj(1, q, [2, 3]),
                ])
            emit_outproj(1, NQC - 1, [0, 1, 2, 3])

    nc.compile()
    return nc


def _get_nc():
    if "nc" not in _cache:
        _cache["nc"] = _build()
    return _cache["nc"]


def kernel(x, Wq, bq, Wk, bk, Wv, bv, Wo, bo):
    from concourse.bass_utils import run_bass_kernel_spmd

    nc = _get_nc()

    x = np.ascontiguousarray(np.asarray(x, dtype=np.float32))
    xt = np.ascontiguousarray(x.reshape(B * S, D).T)          # [D, B*S]
    idt = np.eye(128, dtype=np.float32)

    def wprep(W, sl, scale=1.0):
        # [1024, 128] slice -> [128 part, KT_PROJ, 128] contiguous f16
        w = np.asarray(W, np.float32)[:, sl] * scale
        return np.ascontiguousarray(
            w.reshape(KT_PROJ, 128, HSLICE).transpose(1, 0, 2)).astype(np.float16)

    in_maps = []
    for c in range(NCORES):
        sl = slice(c * HSLICE, (c + 1) * HSLICE)
        in_maps.append({
            "xt": xt.astype(np.float16),
            "wq": wprep(Wq, sl, 1.0 / 8.0),
            "wk": wprep(Wk, sl),
            "wv": wprep(Wv, sl),
            "bq": (np.asarray(bq, np.float32)[sl] / 8.0).reshape(HSLICE, 1),
            "bk": np.asarray(bk, np.float32)[sl].reshape(HSLICE, 1),
            "bv": np.asarray(bv, np.float32)[sl].reshape(HSLICE, 1),
            "wo": np.ascontiguousarray(np.asarray(Wo, np.float32)[sl, :]).astype(np.float16),
            "idt": idt,
        })

    res = run_bass_kernel_spmd(nc, in_maps, core_ids=list(range(NCORES)),
                               trace=bool(int(os.environ.get("KTRACE", "0"))))
    _cache["last_result"] = res
    acc = res.results[0]["out"].astype(np.float32)
    for c in range(1, NCORES):
        acc += res.results[c]["out"].astype(np.float32)
    acc += np.asarray(bo, np.float32)[None, :]
    return acc.reshape(B, S, D)
